# revision 15
# baseline (speedup 1.0000x reference)
"""Trainium2 Bass kernel for the DecoderSRNN step (data-parallel over 8 cores).

Contract: kernel(**inputs) takes the FULL unsharded inputs (as produced by
reference.setup_inputs()) and returns the FULL outputs matching
reference.reference(**inputs): (log_softmax_output, new_hidden, top1,
new_stacks).
"""

import numpy as np

import concourse.bass as bass
import concourse.bacc as bacc
import concourse.mybir as mybir
import concourse.tile as tile
from concourse.bass_utils import run_bass_kernel_spmd

# Problem dims (hardcoded per the grading contract).
B, H, V, NS, S, D, E = 4096, 256, 32000, 2, 128, 4, 64
NCORES = 8
BL = B // NCORES          # 512 batch rows per core
P = 128                   # partitions
KC = H // P               # 2 contraction chunks over H
NBC = BL // P             # 4 batch chunks per core
VT = 500                  # logits tile width (fits one PSUM bank)
NVH = 32                  # logits tiles per half
VH = VT * NVH             # 16000 (half of V)

_f32 = mybir.dt.float32
_i32 = mybir.dt.int32
_u32 = mybir.dt.uint32
AF = mybir.ActivationFunctionType
ALU = mybir.AluOpType

_PROG_CACHE = {}


def _build_program(with_bout: bool, stages=(1, 2, 3)):
    nc = bacc.Bacc(
        "TRN2", target_bir_lowering=False, debug=False, num_devices=NCORES
    )

    def din(name, shape, dt=_f32):
        return nc.dram_tensor(name, shape, dt, kind="ExternalInput").ap()

    def dout(name, shape, dt=_f32):
        return nc.dram_tensor(name, shape, dt, kind="ExternalOutput").ap()

    ids = din("ids", [BL, 1], _i32)
    hidT = din("hidT", [P, KC, BL])
    stk = din("stk", [BL, NS, S, E])
    embt = din("embt", [V, H])
    winT = din("winT", [P, KC, H])
    whhT = din("whhT", [P, KC, H])
    ws2hT = din("ws2hT", [P, NS, KC, H])
    wactT = din("wactT", [P, KC, NS * 3])
    wpushT = din("wpushT", [P, KC, NS * E])
    woutT = din("woutT", [P, KC, V])
    midb = din("midb", [P, KC])
    ba = din("ba", [NS * 3, 1])
    bp = din("bp", [NS * E, 1])
    emptyr = din("emptyr", [1, E])
    identc = din("identc", [P, P])
    identstk = din("identstk", [P, E])
    if with_bout:
        boutd = din("boutd", [1, V])

    out_ls = dout("out_ls", [BL, V])
    out_nh = dout("out_nh", [BL, H])
    out_top1 = dout("out_top1", [BL, 1], _i32)
    out_stk = dout("out_stk", [BL, NS, S, E])

    with tile.TileContext(nc) as tc:
        with tc.tile_pool(name="persist", bufs=1) as pp:
            ident = pp.tile([P, P], _f32)
            nc.sync.dma_start(out=ident[:], in_=identc[:])
            identst = pp.tile([P, E], _f32)
            nc.sync.dma_start(out=identst[:], in_=identstk[:])

            w_in = pp.tile([P, KC, H], _f32)
            nc.sync.dma_start(out=w_in[:], in_=winT[:])
            w_hh = pp.tile([P, KC, H], _f32)
            nc.sync.dma_start(out=w_hh[:], in_=whhT[:])
            w_s2h = pp.tile([P, NS, KC, H], _f32)
            nc.sync.dma_start(out=w_s2h[:], in_=ws2hT[:])
            w_act = pp.tile([P, KC, NS * 3], _f32)
            nc.sync.dma_start(out=w_act[:], in_=wactT[:])
            w_push = pp.tile([P, KC, NS * E], _f32)
            nc.sync.dma_start(out=w_push[:], in_=wpushT[:])
            mb = pp.tile([P, KC], _f32)
            nc.sync.dma_start(out=mb[:], in_=midb[:])
            bat = pp.tile([NS * 3, 1], _f32)
            nc.sync.dma_start(out=bat[:], in_=ba[:])
            bpt = pp.tile([NS * E, 1], _f32)
            nc.sync.dma_start(out=bpt[:], in_=bp[:])
            emp = pp.tile([P, E], _f32)
            nc.sync.dma_start(out=emp[:], in_=emptyr.to_broadcast([P, E]))
            hid = pp.tile([P, KC, BL], _f32)
            nc.sync.dma_start(out=hid[:], in_=hidT[:])
            if with_bout:
                ones1 = pp.tile([1, P], _f32)
                nc.vector.memset(ones1[:], 1.0)
                boutt = pp.tile([1, V], _f32)
                nc.sync.dma_start(out=boutt[:], in_=boutd[:])

            offq = pp.tile([P, 4], _f32)
            for _q in range(4):
                nc.vector.memset(offq[:, _q:_q + 1], float(_q * 8000))
            nhT = pp.tile([P, KC, BL], _f32)
            actsb = pp.tile([P, NBC, NS * 3], _f32)
            pvb = pp.tile([P, NBC, NS, E], _f32)

            # ---------------- stage 1: RNN cell ----------------
            with tc.tile_pool(name="s1", bufs=2) as s1, \
                 tc.tile_pool(name="ps1", bufs=2, space="PSUM") as ps1:
                embT = s1.tile([P, KC, BL], _f32, bufs=1)
                svT = s1.tile([P, NS, KC, BL], _f32, bufs=1)
                for c in range(NBC):
                    idst = s1.tile([P, 1], _i32)
                    nc.sync.dma_start(out=idst[:], in_=ids[c * P:(c + 1) * P, :])
                    embg = s1.tile([P, H], _f32)
                    nc.gpsimd.indirect_dma_start(
                        out=embg[:], out_offset=None, in_=embt[:],
                        in_offset=bass.IndirectOffsetOnAxis(ap=idst[:, 0:1], axis=0),
                    )
                    svg = s1.tile([P, NS, D * E], _f32)
                    nc.sync.dma_start(
                        out=svg[:],
                        in_=stk[c * P:(c + 1) * P, :, 0:D, :].rearrange(
                            "p n d e -> p n (d e)"),
                    )
                    for kc in range(KC):
                        trp = ps1.tile([P, P], _f32, tag="tr", name="trp")
                        nc.tensor.transpose(
                            out=trp[:], in_=embg[:, kc * P:(kc + 1) * P],
                            identity=ident[:])
                        nc.vector.tensor_copy(
                            out=embT[:, kc, c * P:(c + 1) * P], in_=trp[:])
                    for n in range(NS):
                        for kc in range(KC):
                            trp = ps1.tile([P, P], _f32, tag="tr", name="trp")
                            nc.tensor.transpose(
                                out=trp[:],
                                in_=svg[:, n, kc * P:(kc + 1) * P],
                                identity=ident[:])
                            nc.vector.tensor_copy(
                                out=svT[:, n, kc, c * P:(c + 1) * P], in_=trp[:])

                # mid = emb@W_in.T + hidden@W_hh.T + stack_readout + biases
                for hc in range(KC):
                    hsl = slice(hc * P, (hc + 1) * P)
                    mps = ps1.tile([P, BL], _f32, tag="mid", name="mps")
                    nc.tensor.matmul(out=mps[:], lhsT=w_in[:, 0, hsl],
                                     rhs=embT[:, 0, :], start=True, stop=False)
                    nc.tensor.matmul(out=mps[:], lhsT=w_in[:, 1, hsl],
                                     rhs=embT[:, 1, :], start=False, stop=False)
                    nc.tensor.matmul(out=mps[:], lhsT=w_hh[:, 0, hsl],
                                     rhs=hid[:, 0, :], start=False, stop=False)
                    nc.tensor.matmul(out=mps[:], lhsT=w_hh[:, 1, hsl],
                                     rhs=hid[:, 1, :], start=False, stop=False)
                    for n in range(NS):
                        for kc in range(KC):
                            last = (n == NS - 1) and (kc == KC - 1)
                            nc.tensor.matmul(
                                out=mps[:], lhsT=w_s2h[:, n, kc, hsl],
                                rhs=svT[:, n, kc, :], start=False, stop=last)
                    nc.scalar.activation(out=nhT[:, hc, :], in_=mps[:],
                                         func=AF.Tanh, bias=mb[:, hc:hc + 1])

                # act logits (push/pop/noop) from OLD hidden
                aps = ps1.tile([NS * 3, BL], _f32, tag="act", name="aps", bufs=1)
                nc.tensor.matmul(out=aps[:], lhsT=w_act[:, 0, :], rhs=hid[:, 0, :],
                                 start=True, stop=False)
                nc.tensor.matmul(out=aps[:], lhsT=w_act[:, 1, :], rhs=hid[:, 1, :],
                                 start=False, stop=True)
                actT = s1.tile([NS * 3, BL], _f32, bufs=1)
                nc.scalar.activation(out=actT[:], in_=aps[:], func=AF.Identity,
                                     bias=bat[:, 0:1])

                # push values tanh(hidden @ W_push.T + b_push)
                pps = ps1.tile([P, BL], _f32, tag="pv", name="pps", bufs=1)
                nc.tensor.matmul(out=pps[:], lhsT=w_push[:, 0, :], rhs=hid[:, 0, :],
                                 start=True, stop=False)
                nc.tensor.matmul(out=pps[:], lhsT=w_push[:, 1, :], rhs=hid[:, 1, :],
                                 start=False, stop=True)
                pvT = s1.tile([P, BL], _f32, bufs=1)
                nc.scalar.activation(out=pvT[:], in_=pps[:], func=AF.Tanh,
                                     bias=bpt[:, 0:1])

                # transpose act/pv/nh back to batch-on-partition layout
                for c in range(NBC):
                    csl = slice(c * P, (c + 1) * P)
                    trp = ps1.tile([P, P], _f32, tag="tr", name="trp")
                    nc.tensor.transpose(out=trp[:, 0:NS * 3],
                                        in_=actT[:, csl],
                                        identity=ident[0:NS * 3, 0:NS * 3])
                    nc.vector.tensor_copy(out=actsb[:, c, :], in_=trp[:, 0:NS * 3])
                    for n in range(NS):
                        trp = ps1.tile([P, P], _f32, tag="tr", name="trp")
                        nc.tensor.transpose(out=trp[:, 0:E],
                                            in_=pvT[n * E:(n + 1) * E, csl],
                                            identity=identst[n * E:(n + 1) * E, :])
                        nc.vector.tensor_copy(out=pvb[:, c, n, :], in_=trp[:, 0:E])
                    nhb = s1.tile([P, H], _f32)
                    for hc in range(KC):
                        trp = ps1.tile([P, P], _f32, tag="tr", name="trp")
                        nc.tensor.transpose(out=trp[:], in_=nhT[:, hc, csl],
                                            identity=ident[:])
                        nc.vector.tensor_copy(out=nhb[:, hc * P:(hc + 1) * P],
                                              in_=trp[:])
                    nc.gpsimd.dma_start(out=out_nh[csl, :], in_=nhb[:])

            # ------- stages 2+3 interleaved: stack update + logits/softmax ----
            # Stage 2 is cut into small s-range units and its emission is
            # interleaved into stage 3's tile loop so its DMA traffic fills
            # the gaps in stage 3's DMA stream.
            SS = 16                    # stack slots per stage-2 unit
            NSS = S // SS              # 8 units per (chunk, stack)
            s2_units = [(c, n, q) for c in range(NBC) for n in range(NS)
                        for q in range(NSS)]
            if 2 not in stages:
                s2_units = []

            with tc.tile_pool(name="s2", bufs=2) as s2, \
                 tc.tile_pool(name="s3big", bufs=2) as s3b, \
                 tc.tile_pool(name="s3w", bufs=3) as s3w, \
                 tc.tile_pool(name="s3s", bufs=2) as s3s, \
                 tc.tile_pool(name="s3o", bufs=2) as s3o, \
                 tc.tile_pool(name="ps3", bufs=8, space="PSUM") as ps3:

                def emit_s2_unit(c, n, q):
                    csl = slice(c * P, (c + 1) * P)
                    push = actsb[:, c, 3 * n + 0:3 * n + 1]
                    pop = actsb[:, c, 3 * n + 1:3 * n + 2]
                    noop = actsb[:, c, 3 * n + 2:3 * n + 3]
                    s_lo = q * SS              # first output slot of unit
                    s_hi = s_lo + SS           # one past last output slot
                    in_lo = max(s_lo - 1, 0)
                    in_hi = min(s_hi + 1, S)
                    n_in = in_hi - in_lo
                    sin = s2.tile([P, SS + 2, E], _f32, tag="sin", name="sin")
                    nc.gpsimd.dma_start(out=sin[:, 0:n_in, :],
                                        in_=stk[csl, n, in_lo:in_hi, :])
                    sinf = sin.rearrange("p s e -> p (s e)")
                    sout = s2.tile([P, SS, E], _f32, tag="sout", name="sout")
                    soutf = sout.rearrange("p s e -> p (s e)")
                    # blended slots within this unit (exclude s=0 and s=S-1)
                    blo = max(s_lo, 1)
                    bhi = min(s_hi, S - 1)
                    nbl = bhi - blo
                    # offsets of the blended range within sin/sout
                    oin = blo - in_lo          # position of slot blo in sin
                    oout = blo - s_lo          # position of slot blo in sout
                    wbl = nbl * E
                    osl = slice(oout * E, oout * E + wbl)
                    dn = slice((oin - 1) * E, (oin - 1) * E + wbl)
                    up = slice((oin + 1) * E, (oin + 1) * E + wbl)
                    mid = slice(oin * E, oin * E + wbl)
                    t2 = s2.tile([P, SS * E], _f32, tag="t2", name="t2", bufs=1)
                    t3 = s2.tile([P, SS * E], _f32, tag="t3", name="t3", bufs=1)
                    nc.scalar.activation(out=soutf[:, osl], in_=sinf[:, dn],
                                         func=AF.Copy, scale=push)
                    nc.vector.tensor_scalar_mul(t2[:, 0:wbl], sinf[:, up], pop)
                    nc.scalar.activation(out=t3[:, 0:wbl], in_=sinf[:, mid],
                                         func=AF.Copy, scale=noop)
                    nc.vector.tensor_tensor(out=soutf[:, osl], in0=soutf[:, osl],
                                            in1=t2[:, 0:wbl], op=ALU.add)
                    nc.vector.tensor_tensor(out=soutf[:, osl], in0=soutf[:, osl],
                                            in1=t3[:, 0:wbl], op=ALU.add)
                    if q == 0:
                        nc.vector.tensor_scalar_mul(soutf[:, 0:E],
                                                    pvb[:, c, n, :], push)
                    if q == NSS - 1:
                        nc.vector.tensor_copy(out=soutf[:, (SS - 1) * E:SS * E],
                                              in_=emp[:])
                    nc.gpsimd.dma_start(out=out_stk[csl, n, s_lo:s_hi, :],
                                        in_=sout[:])

                s2_iter = iter(s2_units)
                NQ = 4                     # quarters per chunk
                QT = (2 * NVH) // NQ       # 16 v-tiles per quarter
                QW = QT * VT               # 8000 cols per quarter
                BIG = float(1 << 16)
                n_s3_tiles = (NBC * 2 * NVH) if 3 in stages else 0
                s2_per_tile = (len(s2_units) / n_s3_tiles) if n_s3_tiles else 0
                s2_credit = [0.0]

                def drip_s2():
                    s2_credit[0] += s2_per_tile
                    while s2_credit[0] >= 1.0:
                        s2_credit[0] -= 1.0
                        u = next(s2_iter, None)
                        if u is not None:
                            emit_s2_unit(*u)

                def emit_finals_quarter(cprev, lgq, nls_prev, q):
                    csl = slice(cprev * P, (cprev + 1) * P)
                    for j0 in range(0, QT, 2):
                        gv0 = q * QW + j0 * VT
                        ot = s3o.tile([P, 2, VT], _f32, tag="ot", name="ot")
                        nc.scalar.activation(
                            out=ot[:, 0, :], in_=lgq[:, j0 * VT:(j0 + 1) * VT],
                            func=AF.Identity, bias=nls_prev[:, 0:1])
                        nc.vector.tensor_scalar_add(
                            ot[:, 1, :], lgq[:, (j0 + 1) * VT:(j0 + 2) * VT],
                            nls_prev[:, 0:1])
                        nc.scalar.dma_start(
                            out=out_ls[csl, gv0:gv0 + 2 * VT],
                            in_=ot.rearrange("p t v -> p (t v)"))

                prev = None
                for c in range(NBC if 3 in stages else 0):
                    csl = slice(c * P, (c + 1) * P)
                    mx = s3s.tile([P, 2 * NVH], _f32, tag="mx", name="mx")
                    sxp = s3s.tile([P, 2 * NVH], _f32, tag="sxp", name="sxp")
                    mq4 = s3s.tile([P, NQ], _f32, tag="mq4", name="mq4")
                    gidx4 = s3s.tile([P, NQ], _f32, tag="gidx4", name="gidx4")
                    quarters = []
                    for q in range(NQ):
                        if prev is not None:
                            emit_finals_quarter(prev[0], prev[1][q], prev[2], q)
                        lg = s3b.tile([P, QW], _f32, tag="lg", name="lg", bufs=4)
                        quarters.append(lg)
                        for j in range(QT):
                            jj = q * QT + j
                            v0 = jj * VT
                            if j % 2 == 0:
                                wt = s3w.tile([P, KC, 2 * VT], _f32, tag="wt",
                                              name="wt", bufs=3)
                                nc.sync.dma_start(
                                    out=wt[:], in_=woutT[:, :, v0:v0 + 2 * VT])
                            wsl = slice((j % 2) * VT, (j % 2) * VT + VT)
                            lps = ps3.tile([P, VT], _f32, tag="lps", name="lps")
                            nc.tensor.matmul(out=lps[:],
                                             lhsT=nhT[:, 0, csl],
                                             rhs=wt[:, 0, wsl],
                                             start=True, stop=False)
                            nc.tensor.matmul(out=lps[:],
                                             lhsT=nhT[:, 1, csl],
                                             rhs=wt[:, 1, wsl],
                                             start=False, stop=not with_bout)
                            if with_bout:
                                nc.tensor.matmul(out=lps[:], lhsT=ones1[:],
                                                 rhs=boutt[:, v0:v0 + VT],
                                                 start=False, stop=True)
                            nc.scalar.activation(out=lg[:, j * VT:(j + 1) * VT],
                                                 in_=lps[:], func=AF.Copy)
                            exps = s3s.tile([P, VT], _f32, tag="exps", name="exps",
                                            bufs=2)
                            nc.scalar.activation(out=exps[:], in_=lps[:],
                                                 func=AF.Exp,
                                                 accum_out=sxp[:, jj:jj + 1])
                            nc.vector.tensor_reduce(out=mx[:, jj:jj + 1],
                                                    in_=lg[:, j * VT:(j + 1) * VT],
                                                    axis=mybir.AxisListType.X,
                                                    op=ALU.max)
                            drip_s2()
                        # per-quarter argmax (first occurrence of quarter max)
                        nc.vector.reduce_max(out=mq4[:, q:q + 1],
                                             in_=mx[:, q * QT:(q + 1) * QT],
                                             axis=mybir.AxisListType.X)
                        im8 = s3s.tile([P, 8], _f32, tag="im8", name="im8")
                        nc.vector.memset(im8[:], -3.4e38)
                        nc.vector.tensor_copy(out=im8[:, 0:1], in_=mq4[:, q:q + 1])
                        idq = s3s.tile([P, 8], _u32, tag="idq", name="idq")
                        nc.vector.max_index(out=idq[:], in_max=im8[:],
                                            in_values=lg[:])
                        nc.vector.tensor_copy(out=gidx4[:, q:q + 1],
                                              in_=idq[:, 0:1])
                    # log-sum-exp (no max subtraction: |logits| is small)
                    st = s3s.tile([P, 1], _f32, tag="st", name="st")
                    nc.vector.reduce_sum(out=st[:], in_=sxp[:],
                                         axis=mybir.AxisListType.X)
                    nls = s3s.tile([P, 1], _f32, tag="nls", name="nls")
                    nc.scalar.activation(out=nls[:], in_=st[:], func=AF.Ln)
                    nc.vector.tensor_scalar_mul(nls[:], nls[:], -1.0)
                    # combine quarter argmaxes: first quarter reaching the max
                    mt = s3s.tile([P, 1], _f32, tag="mt", name="mt")
                    nc.vector.reduce_max(out=mt[:], in_=mq4[:],
                                         axis=mybir.AxisListType.X)
                    # global idx per quarter = idx_q + q*QW (exact in f32)
                    nc.vector.tensor_tensor(out=gidx4[:], in0=gidx4[:],
                                            in1=offq[:], op=ALU.add)
                    msk4 = s3s.tile([P, NQ], _f32, tag="msk4", name="msk4")
                    nc.vector.tensor_scalar(out=msk4[:], in0=mq4[:],
                                            scalar1=mt[:, 0:1], scalar2=None,
                                            op0=ALU.is_ge)
                    # cand = mask*(gidx-BIG) + BIG  -> gidx where mask else BIG
                    nc.vector.tensor_scalar_add(gidx4[:], gidx4[:], -BIG)
                    nc.vector.tensor_tensor(out=gidx4[:], in0=gidx4[:],
                                            in1=msk4[:], op=ALU.mult)
                    nc.vector.tensor_scalar_add(gidx4[:], gidx4[:], BIG)
                    idxf = s3s.tile([P, 1], _f32, tag="idxf", name="idxf")
                    nc.vector.tensor_reduce(out=idxf[:], in_=gidx4[:],
                                            axis=mybir.AxisListType.X,
                                            op=ALU.min)
                    t1 = s3s.tile([P, 1], _i32, tag="t1", name="t1")
                    nc.vector.tensor_copy(out=t1[:], in_=idxf[:])
                    nc.gpsimd.dma_start(out=out_top1[csl, :], in_=t1[:])
                    prev = (c, quarters, nls)
                if prev is not None:
                    for q in range(NQ):
                        emit_finals_quarter(prev[0], prev[1][q], prev[2], q)
                # drain any remaining stage-2 units
                for u in s2_iter:
                    emit_s2_unit(*u)

    return _finish(nc)


def _finish(nc):
    nc.compile()
    return nc


def _get_program(with_bout: bool):
    key = bool(with_bout)
    if key not in _PROG_CACHE:
        _PROG_CACHE[key] = _build_program(key)
    return _PROG_CACHE[key]


def _f(x):
    return np.ascontiguousarray(np.asarray(x, dtype=np.float32))


def _prep_inputs(input_ids, hidden, stacks, emb_table, W_in, b_in, W_hh, b_hh,
                 W_s2h, b_s2h, W_act, b_act, W_push, b_push, W_out, b_out,
                 empty_elem):
    ids = np.ascontiguousarray(np.asarray(input_ids).reshape(B).astype(np.int32))
    hidden = _f(hidden)
    stacks = _f(stacks)
    embt = _f(emb_table)
    W_in, b_in, W_hh, b_hh = _f(W_in), _f(b_in), _f(W_hh), _f(b_hh)
    W_s2h, b_s2h = _f(W_s2h), _f(b_s2h)
    W_act, b_act = _f(W_act), _f(b_act)
    W_push, b_push = _f(W_push), _f(b_push)
    W_out, b_out = _f(W_out), _f(b_out)
    empty_elem = _f(empty_elem)

    def kc_major(a):  # [H, X] -> [P, KC, X]
        return np.ascontiguousarray(
            a.reshape(KC, P, *a.shape[1:]).transpose(1, 0, *range(2, a.ndim + 1)))

    winT = kc_major(W_in.T)                                   # [P, KC, H]
    whhT = kc_major(W_hh.T)
    # einsum('bnd,nhd->bh'): lhsT[d, h] = W_s2h[n, h, d]
    ws2hT = np.ascontiguousarray(
        W_s2h.transpose(0, 2, 1)                              # [NS, D*E, H]
        .reshape(NS, KC, P, H).transpose(2, 0, 1, 3))         # [P, NS, KC, H]
    wactT = kc_major(W_act.transpose(2, 0, 1).reshape(H, NS * 3))
    wpushT = kc_major(W_push.transpose(2, 0, 1).reshape(H, NS * E))
    woutT = kc_major(W_out.T)                                 # [P, KC, V]
    midb = np.ascontiguousarray(
        (b_in + b_hh + b_s2h.sum(0)).reshape(KC, P).T)        # [P, KC]
    ba = np.ascontiguousarray(b_act.reshape(NS * 3, 1))
    bp = np.ascontiguousarray(b_push.reshape(NS * E, 1))
    emptyr = np.ascontiguousarray(empty_elem.reshape(1, E))
    with_bout = bool(np.any(b_out))

    shared = dict(embt=embt, winT=winT, whhT=whhT, ws2hT=ws2hT, wactT=wactT,
                  wpushT=wpushT, woutT=woutT, midb=midb, ba=ba, bp=bp,
                  emptyr=emptyr,
                  identc=np.eye(P, dtype=np.float32),
                  identstk=np.ascontiguousarray(
                      np.tile(np.eye(E, dtype=np.float32), (2, 1))))
    if with_bout:
        shared["boutd"] = np.ascontiguousarray(b_out.reshape(1, V))

    hidT_full = hidden.T                                      # [H, B] view
    in_maps = []
    for c in range(NCORES):
        b0 = c * BL
        m = dict(shared)
        m["ids"] = np.ascontiguousarray(ids[b0:b0 + BL].reshape(BL, 1))
        m["hidT"] = np.ascontiguousarray(
            hidT_full[:, b0:b0 + BL].reshape(KC, P, BL).transpose(1, 0, 2))
        m["stk"] = np.ascontiguousarray(stacks[b0:b0 + BL])
        in_maps.append(m)
    return in_maps, with_bout


def _assemble(results, ids_dtype):
    out_ls = np.concatenate([r["out_ls"] for r in results], axis=0)
    new_hidden = np.concatenate([r["out_nh"] for r in results], axis=0)
    top1 = np.concatenate([r["out_top1"] for r in results], axis=0)
    new_stacks = np.concatenate([r["out_stk"] for r in results], axis=0)
    if np.dtype(ids_dtype).itemsize == 8:
        top1 = top1.astype(np.int64)
    return out_ls, new_hidden, top1, new_stacks


def kernel(input_ids, hidden, stacks, emb_table, W_in, b_in, W_hh, b_hh,
           W_s2h, b_s2h, W_act, b_act, W_push, b_push, W_out, b_out,
           empty_elem):
    ids_dtype = np.asarray(input_ids).dtype
    in_maps, with_bout = _prep_inputs(
        input_ids, hidden, stacks, emb_table, W_in, b_in, W_hh, b_hh,
        W_s2h, b_s2h, W_act, b_act, W_push, b_push, W_out, b_out, empty_elem)
    nc = _get_program(with_bout)
    res = run_bass_kernel_spmd(nc, in_maps, list(range(NCORES)))
    return _assemble(res.results, ids_dtype)


# revision 17
# speedup vs baseline: 11.6954x; 11.6954x over previous
"""Trainium2 Bass kernel for the DecoderSRNN step (data-parallel over 8 cores).

Contract: kernel(**inputs) takes the FULL unsharded inputs (as produced by
reference.setup_inputs()) and returns the FULL outputs matching
reference.reference(**inputs): (log_softmax_output, new_hidden, top1,
new_stacks).
"""

import numpy as np

import concourse.bass as bass
import concourse.bacc as bacc
import concourse.mybir as mybir
import concourse.tile as tile
from concourse.bass_utils import run_bass_kernel_spmd

# Problem dims (hardcoded per the grading contract).
B, H, V, NS, S, D, E = 4096, 256, 32000, 2, 128, 4, 64
NCORES = 8
BL = B // NCORES          # 512 batch rows per core
P = 128                   # partitions
KC = H // P               # 2 contraction chunks over H
NBC = BL // P             # 4 batch chunks per core
VT = 500                  # logits tile width (fits one PSUM bank)
NVH = 32                  # logits tiles per half
VH = VT * NVH             # 16000 (half of V)

_f32 = mybir.dt.float32
_i32 = mybir.dt.int32
_u32 = mybir.dt.uint32
AF = mybir.ActivationFunctionType
ALU = mybir.AluOpType

_PROG_CACHE = {}


def _build_program(with_bout: bool, stages=(1, 2, 3)):
    nc = bacc.Bacc(
        "TRN2", target_bir_lowering=False, debug=False, num_devices=NCORES
    )

    def din(name, shape, dt=_f32):
        return nc.dram_tensor(name, shape, dt, kind="ExternalInput").ap()

    def dout(name, shape, dt=_f32):
        return nc.dram_tensor(name, shape, dt, kind="ExternalOutput").ap()

    ids = din("ids", [BL, 1], _i32)
    hidT = din("hidT", [P, KC, BL])
    stk = din("stk", [BL, NS, S, E])
    embt = din("embt", [V, H])
    winT = din("winT", [P, KC, H])
    whhT = din("whhT", [P, KC, H])
    ws2hT = din("ws2hT", [P, NS, KC, H])
    wactT = din("wactT", [P, KC, NS * 3])
    wpushT = din("wpushT", [P, KC, NS * E])
    woutT = din("woutT", [P, KC, V])
    midb = din("midb", [P, KC])
    ba = din("ba", [NS * 3, 1])
    bp = din("bp", [NS * E, 1])
    emptyr = din("emptyr", [1, E])
    identc = din("identc", [P, P])
    identstk = din("identstk", [P, E])
    if with_bout:
        boutd = din("boutd", [1, V])

    out_ls = dout("out_ls", [BL, V])
    out_nh = dout("out_nh", [BL, H])
    out_top1 = dout("out_top1", [BL, 1], _i32)
    out_stk = dout("out_stk", [BL, NS, S, E])

    with tile.TileContext(nc) as tc:
        with tc.tile_pool(name="persist", bufs=1) as pp:
            ident = pp.tile([P, P], _f32)
            nc.sync.dma_start(out=ident[:], in_=identc[:])
            identst = pp.tile([P, E], _f32)
            nc.sync.dma_start(out=identst[:], in_=identstk[:])

            w_in = pp.tile([P, KC, H], _f32)
            nc.sync.dma_start(out=w_in[:], in_=winT[:])
            w_hh = pp.tile([P, KC, H], _f32)
            nc.sync.dma_start(out=w_hh[:], in_=whhT[:])
            w_s2h = pp.tile([P, NS, KC, H], _f32)
            nc.sync.dma_start(out=w_s2h[:], in_=ws2hT[:])
            w_act = pp.tile([P, KC, NS * 3], _f32)
            nc.sync.dma_start(out=w_act[:], in_=wactT[:])
            w_push = pp.tile([P, KC, NS * E], _f32)
            nc.sync.dma_start(out=w_push[:], in_=wpushT[:])
            mb = pp.tile([P, KC], _f32)
            nc.sync.dma_start(out=mb[:], in_=midb[:])
            bat = pp.tile([NS * 3, 1], _f32)
            nc.sync.dma_start(out=bat[:], in_=ba[:])
            bpt = pp.tile([NS * E, 1], _f32)
            nc.sync.dma_start(out=bpt[:], in_=bp[:])
            emp = pp.tile([P, E], _f32)
            nc.sync.dma_start(out=emp[:], in_=emptyr.to_broadcast([P, E]))

            if with_bout:
                ones1 = pp.tile([1, P], _f32)
                nc.vector.memset(ones1[:], 1.0)
                boutt = pp.tile([1, V], _f32)
                nc.sync.dma_start(out=boutt[:], in_=boutd[:])

            offq = pp.tile([P, 4], _f32)
            for _q in range(4):
                nc.vector.memset(offq[:, _q:_q + 1], float(_q * 8000))
            nhT = pp.tile([P, KC, BL], _f32)
            actsb = pp.tile([P, NBC, NS * 3], _f32)
            pvb = pp.tile([P, NBC, NS, E], _f32)

            # ---------------- stage 1: RNN cell ----------------
            with tc.tile_pool(name="s1", bufs=2) as s1, \
                 tc.tile_pool(name="ps1", bufs=2, space="PSUM") as ps1:
                hid = s1.tile([P, KC, BL], _f32, bufs=1)
                nc.sync.dma_start(out=hid[:], in_=hidT[:])
                embT = s1.tile([P, KC, BL], _f32, bufs=1)
                svT = s1.tile([P, NS, KC, BL], _f32, bufs=1)
                for c in range(NBC):
                    idst = s1.tile([P, 1], _i32)
                    nc.sync.dma_start(out=idst[:], in_=ids[c * P:(c + 1) * P, :])
                    embg = s1.tile([P, H], _f32)
                    nc.gpsimd.indirect_dma_start(
                        out=embg[:], out_offset=None, in_=embt[:],
                        in_offset=bass.IndirectOffsetOnAxis(ap=idst[:, 0:1], axis=0),
                    )
                    svg = s1.tile([P, NS, D * E], _f32)
                    nc.sync.dma_start(
                        out=svg[:],
                        in_=stk[c * P:(c + 1) * P, :, 0:D, :].rearrange(
                            "p n d e -> p n (d e)"),
                    )
                    for kc in range(KC):
                        trp = ps1.tile([P, P], _f32, tag="tr", name="trp")
                        nc.tensor.transpose(
                            out=trp[:], in_=embg[:, kc * P:(kc + 1) * P],
                            identity=ident[:])
                        nc.vector.tensor_copy(
                            out=embT[:, kc, c * P:(c + 1) * P], in_=trp[:])
                    for n in range(NS):
                        for kc in range(KC):
                            trp = ps1.tile([P, P], _f32, tag="tr", name="trp")
                            nc.tensor.transpose(
                                out=trp[:],
                                in_=svg[:, n, kc * P:(kc + 1) * P],
                                identity=ident[:])
                            nc.vector.tensor_copy(
                                out=svT[:, n, kc, c * P:(c + 1) * P], in_=trp[:])

                # mid = emb@W_in.T + hidden@W_hh.T + stack_readout + biases
                for hc in range(KC):
                    hsl = slice(hc * P, (hc + 1) * P)
                    mps = ps1.tile([P, BL], _f32, tag="mid", name="mps")
                    nc.tensor.matmul(out=mps[:], lhsT=w_in[:, 0, hsl],
                                     rhs=embT[:, 0, :], start=True, stop=False)
                    nc.tensor.matmul(out=mps[:], lhsT=w_in[:, 1, hsl],
                                     rhs=embT[:, 1, :], start=False, stop=False)
                    nc.tensor.matmul(out=mps[:], lhsT=w_hh[:, 0, hsl],
                                     rhs=hid[:, 0, :], start=False, stop=False)
                    nc.tensor.matmul(out=mps[:], lhsT=w_hh[:, 1, hsl],
                                     rhs=hid[:, 1, :], start=False, stop=False)
                    for n in range(NS):
                        for kc in range(KC):
                            last = (n == NS - 1) and (kc == KC - 1)
                            nc.tensor.matmul(
                                out=mps[:], lhsT=w_s2h[:, n, kc, hsl],
                                rhs=svT[:, n, kc, :], start=False, stop=last)
                    nc.scalar.activation(out=nhT[:, hc, :], in_=mps[:],
                                         func=AF.Tanh, bias=mb[:, hc:hc + 1])

                # act logits (push/pop/noop) from OLD hidden
                aps = ps1.tile([NS * 3, BL], _f32, tag="act", name="aps", bufs=1)
                nc.tensor.matmul(out=aps[:], lhsT=w_act[:, 0, :], rhs=hid[:, 0, :],
                                 start=True, stop=False)
                nc.tensor.matmul(out=aps[:], lhsT=w_act[:, 1, :], rhs=hid[:, 1, :],
                                 start=False, stop=True)
                actT = s1.tile([NS * 3, BL], _f32, bufs=1)
                nc.scalar.activation(out=actT[:], in_=aps[:], func=AF.Identity,
                                     bias=bat[:, 0:1])

                # push values tanh(hidden @ W_push.T + b_push)
                pps = ps1.tile([P, BL], _f32, tag="pv", name="pps", bufs=1)
                nc.tensor.matmul(out=pps[:], lhsT=w_push[:, 0, :], rhs=hid[:, 0, :],
                                 start=True, stop=False)
                nc.tensor.matmul(out=pps[:], lhsT=w_push[:, 1, :], rhs=hid[:, 1, :],
                                 start=False, stop=True)
                pvT = s1.tile([P, BL], _f32, bufs=1)
                nc.scalar.activation(out=pvT[:], in_=pps[:], func=AF.Tanh,
                                     bias=bpt[:, 0:1])

                # transpose act/pv/nh back to batch-on-partition layout
                for c in range(NBC):
                    csl = slice(c * P, (c + 1) * P)
                    trp = ps1.tile([P, P], _f32, tag="tr", name="trp")
                    nc.tensor.transpose(out=trp[:, 0:NS * 3],
                                        in_=actT[:, csl],
                                        identity=ident[0:NS * 3, 0:NS * 3])
                    nc.vector.tensor_copy(out=actsb[:, c, :], in_=trp[:, 0:NS * 3])
                    for n in range(NS):
                        trp = ps1.tile([P, P], _f32, tag="tr", name="trp")
                        nc.tensor.transpose(out=trp[:, 0:E],
                                            in_=pvT[n * E:(n + 1) * E, csl],
                                            identity=identst[n * E:(n + 1) * E, :])
                        nc.vector.tensor_copy(out=pvb[:, c, n, :], in_=trp[:, 0:E])
                    nhb = s1.tile([P, H], _f32)
                    for hc in range(KC):
                        trp = ps1.tile([P, P], _f32, tag="tr", name="trp")
                        nc.tensor.transpose(out=trp[:], in_=nhT[:, hc, csl],
                                            identity=ident[:])
                        nc.vector.tensor_copy(out=nhb[:, hc * P:(hc + 1) * P],
                                              in_=trp[:])
                    nc.gpsimd.dma_start(out=out_nh[csl, :], in_=nhb[:])

            # ------- stages 2+3 interleaved: stack update + logits/softmax ----
            # Stage 2 is cut into small s-range units and its emission is
            # interleaved into stage 3's tile loop so its DMA traffic fills
            # the gaps in stage 3's DMA stream.
            SS = 16                    # stack slots per stage-2 unit
            NSS = S // SS              # 8 units per (chunk, stack)
            s2_units = [(c, n, q) for c in range(NBC) for n in range(NS)
                        for q in range(NSS)]
            if 2 not in stages:
                s2_units = []

            with tc.tile_pool(name="s2", bufs=2) as s2, \
                 tc.tile_pool(name="s3big", bufs=2) as s3b, \
                 tc.tile_pool(name="s3w", bufs=3) as s3w, \
                 tc.tile_pool(name="s3s", bufs=2) as s3s, \
                 tc.tile_pool(name="s3o", bufs=2) as s3o, \
                 tc.tile_pool(name="ps3", bufs=8, space="PSUM") as ps3:

                def emit_s2_unit(c, n, q):
                    csl = slice(c * P, (c + 1) * P)
                    push = actsb[:, c, 3 * n + 0:3 * n + 1]
                    pop = actsb[:, c, 3 * n + 1:3 * n + 2]
                    noop = actsb[:, c, 3 * n + 2:3 * n + 3]
                    s_lo = q * SS              # first output slot of unit
                    s_hi = s_lo + SS           # one past last output slot
                    in_lo = max(s_lo - 1, 0)
                    in_hi = min(s_hi + 1, S)
                    n_in = in_hi - in_lo
                    sin = s2.tile([P, SS + 2, E], _f32, tag="sin", name="sin")
                    nc.gpsimd.dma_start(out=sin[:, 0:n_in, :],
                                        in_=stk[csl, n, in_lo:in_hi, :])
                    sinf = sin.rearrange("p s e -> p (s e)")
                    sout = s2.tile([P, SS, E], _f32, tag="sout", name="sout")
                    soutf = sout.rearrange("p s e -> p (s e)")
                    # blended slots within this unit (exclude s=0 and s=S-1)
                    blo = max(s_lo, 1)
                    bhi = min(s_hi, S - 1)
                    nbl = bhi - blo
                    # offsets of the blended range within sin/sout
                    oin = blo - in_lo          # position of slot blo in sin
                    oout = blo - s_lo          # position of slot blo in sout
                    wbl = nbl * E
                    osl = slice(oout * E, oout * E + wbl)
                    dn = slice((oin - 1) * E, (oin - 1) * E + wbl)
                    up = slice((oin + 1) * E, (oin + 1) * E + wbl)
                    mid = slice(oin * E, oin * E + wbl)
                    t2 = s2.tile([P, SS * E], _f32, tag="t2", name="t2", bufs=1)
                    t3 = s2.tile([P, SS * E], _f32, tag="t3", name="t3", bufs=1)
                    nc.scalar.activation(out=soutf[:, osl], in_=sinf[:, dn],
                                         func=AF.Copy, scale=push)
                    nc.vector.tensor_scalar_mul(t2[:, 0:wbl], sinf[:, up], pop)
                    nc.scalar.activation(out=t3[:, 0:wbl], in_=sinf[:, mid],
                                         func=AF.Copy, scale=noop)
                    nc.vector.tensor_tensor(out=soutf[:, osl], in0=soutf[:, osl],
                                            in1=t2[:, 0:wbl], op=ALU.add)
                    nc.vector.tensor_tensor(out=soutf[:, osl], in0=soutf[:, osl],
                                            in1=t3[:, 0:wbl], op=ALU.add)
                    if q == 0:
                        nc.vector.tensor_scalar_mul(soutf[:, 0:E],
                                                    pvb[:, c, n, :], push)
                    if q == NSS - 1:
                        nc.vector.tensor_copy(out=soutf[:, (SS - 1) * E:SS * E],
                                              in_=emp[:])
                    nc.gpsimd.dma_start(out=out_stk[csl, n, s_lo:s_hi, :],
                                        in_=sout[:])

                s2_iter = iter(s2_units)
                NQ = 4                     # quarters per chunk
                QT = (2 * NVH) // NQ       # 16 v-tiles per quarter
                QW = QT * VT               # 8000 cols per quarter
                BIG = float(1 << 16)
                n_s3_tiles = (NBC * 2 * NVH) if 3 in stages else 0
                s2_per_tile = (0.8 * len(s2_units) / n_s3_tiles) if n_s3_tiles else 0
                s2_credit = [0.0]

                def drip_s2():
                    s2_credit[0] += s2_per_tile
                    while s2_credit[0] >= 1.0:
                        s2_credit[0] -= 1.0
                        u = next(s2_iter, None)
                        if u is not None:
                            emit_s2_unit(*u)

                def emit_finals_quarter(cprev, lgq, nls_prev, q):
                    csl = slice(cprev * P, (cprev + 1) * P)
                    for j0 in range(0, QT, 4):
                        gv0 = q * QW + j0 * VT
                        ot = s3o.tile([P, 4, VT], _f32, tag="ot", name="ot")
                        nc.scalar.activation(
                            out=ot[:, 0:2, :].rearrange("p t v -> p (t v)"),
                            in_=lgq[:, j0 * VT:(j0 + 2) * VT],
                            func=AF.Identity, bias=nls_prev[:, 0:1])
                        nc.vector.tensor_scalar_add(
                            ot[:, 2:4, :].rearrange("p t v -> p (t v)"),
                            lgq[:, (j0 + 2) * VT:(j0 + 4) * VT],
                            nls_prev[:, 0:1])
                        nc.scalar.dma_start(
                            out=out_ls[csl, gv0:gv0 + 4 * VT],
                            in_=ot.rearrange("p t v -> p (t v)"))

                prev = None
                for c in range(NBC if 3 in stages else 0):
                    csl = slice(c * P, (c + 1) * P)
                    mx = s3s.tile([P, 2 * NVH], _f32, tag="mx", name="mx")
                    sxp = s3s.tile([P, 2 * NVH], _f32, tag="sxp", name="sxp")
                    mq4 = s3s.tile([P, NQ], _f32, tag="mq4", name="mq4")
                    gidx4 = s3s.tile([P, NQ], _f32, tag="gidx4", name="gidx4")
                    quarters = []
                    for q in range(NQ):
                        if prev is not None:
                            emit_finals_quarter(prev[0], prev[1][q], prev[2], q)
                        lg = s3b.tile([P, QW], _f32, tag="lg", name="lg", bufs=4)
                        quarters.append(lg)
                        for j in range(QT):
                            jj = q * QT + j
                            v0 = jj * VT
                            if j % 2 == 0:
                                wt = s3w.tile([P, KC, 2 * VT], _f32, tag="wt",
                                              name="wt", bufs=3)
                                nc.sync.dma_start(
                                    out=wt[:], in_=woutT[:, :, v0:v0 + 2 * VT])
                            wsl = slice((j % 2) * VT, (j % 2) * VT + VT)
                            lps = ps3.tile([P, VT], _f32, tag="lps", name="lps")
                            nc.tensor.matmul(out=lps[:],
                                             lhsT=nhT[:, 0, csl],
                                             rhs=wt[:, 0, wsl],
                                             start=True, stop=False)
                            nc.tensor.matmul(out=lps[:],
                                             lhsT=nhT[:, 1, csl],
                                             rhs=wt[:, 1, wsl],
                                             start=False, stop=not with_bout)
                            if with_bout:
                                nc.tensor.matmul(out=lps[:], lhsT=ones1[:],
                                                 rhs=boutt[:, v0:v0 + VT],
                                                 start=False, stop=True)
                            nc.scalar.activation(out=lg[:, j * VT:(j + 1) * VT],
                                                 in_=lps[:], func=AF.Copy)
                            nc.scalar.activation(out=lps[:], in_=lps[:],
                                                 func=AF.Exp,
                                                 accum_out=sxp[:, jj:jj + 1])
                            nc.vector.tensor_reduce(out=mx[:, jj:jj + 1],
                                                    in_=lg[:, j * VT:(j + 1) * VT],
                                                    axis=mybir.AxisListType.X,
                                                    op=ALU.max)
                            drip_s2()
                        # per-quarter argmax (first occurrence of quarter max)
                        nc.vector.reduce_max(out=mq4[:, q:q + 1],
                                             in_=mx[:, q * QT:(q + 1) * QT],
                                             axis=mybir.AxisListType.X)
                        im8 = s3s.tile([P, 8], _f32, tag="im8", name="im8")
                        nc.vector.memset(im8[:], -3.4e38)
                        nc.vector.tensor_copy(out=im8[:, 0:1], in_=mq4[:, q:q + 1])
                        idq = s3s.tile([P, 8], _u32, tag="idq", name="idq")
                        nc.vector.max_index(out=idq[:], in_max=im8[:],
                                            in_values=lg[:])
                        nc.vector.tensor_copy(out=gidx4[:, q:q + 1],
                                              in_=idq[:, 0:1])
                    # log-sum-exp (no max subtraction: |logits| is small)
                    st = s3s.tile([P, 1], _f32, tag="st", name="st")
                    nc.vector.reduce_sum(out=st[:], in_=sxp[:],
                                         axis=mybir.AxisListType.X)
                    nls = s3s.tile([P, 1], _f32, tag="nls", name="nls")
                    nc.scalar.activation(out=nls[:], in_=st[:], func=AF.Ln)
                    nc.vector.tensor_scalar_mul(nls[:], nls[:], -1.0)
                    # combine quarter argmaxes: first quarter reaching the max
                    mt = s3s.tile([P, 1], _f32, tag="mt", name="mt")
                    nc.vector.reduce_max(out=mt[:], in_=mq4[:],
                                         axis=mybir.AxisListType.X)
                    # global idx per quarter = idx_q + q*QW (exact in f32)
                    nc.vector.tensor_tensor(out=gidx4[:], in0=gidx4[:],
                                            in1=offq[:], op=ALU.add)
                    msk4 = s3s.tile([P, NQ], _f32, tag="msk4", name="msk4")
                    nc.vector.tensor_scalar(out=msk4[:], in0=mq4[:],
                                            scalar1=mt[:, 0:1], scalar2=None,
                                            op0=ALU.is_ge)
                    # cand = mask*(gidx-BIG) + BIG  -> gidx where mask else BIG
                    nc.vector.tensor_scalar_add(gidx4[:], gidx4[:], -BIG)
                    nc.vector.tensor_tensor(out=gidx4[:], in0=gidx4[:],
                                            in1=msk4[:], op=ALU.mult)
                    nc.vector.tensor_scalar_add(gidx4[:], gidx4[:], BIG)
                    idxf = s3s.tile([P, 1], _f32, tag="idxf", name="idxf")
                    nc.vector.tensor_reduce(out=idxf[:], in_=gidx4[:],
                                            axis=mybir.AxisListType.X,
                                            op=ALU.min)
                    t1 = s3s.tile([P, 1], _i32, tag="t1", name="t1")
                    nc.vector.tensor_copy(out=t1[:], in_=idxf[:])
                    nc.gpsimd.dma_start(out=out_top1[csl, :], in_=t1[:])
                    prev = (c, quarters, nls)
                if prev is not None:
                    for q in range(NQ):
                        emit_finals_quarter(prev[0], prev[1][q], prev[2], q)
                # drain any remaining stage-2 units
                for u in s2_iter:
                    emit_s2_unit(*u)

    return _finish(nc)


def _finish(nc):
    nc.compile()
    return nc


def _get_program(with_bout: bool):
    key = bool(with_bout)
    if key not in _PROG_CACHE:
        _PROG_CACHE[key] = _build_program(key)
    return _PROG_CACHE[key]


def _f(x):
    return np.ascontiguousarray(np.asarray(x, dtype=np.float32))


def _prep_inputs(input_ids, hidden, stacks, emb_table, W_in, b_in, W_hh, b_hh,
                 W_s2h, b_s2h, W_act, b_act, W_push, b_push, W_out, b_out,
                 empty_elem):
    ids = np.ascontiguousarray(np.asarray(input_ids).reshape(B).astype(np.int32))
    hidden = _f(hidden)
    stacks = _f(stacks)
    embt = _f(emb_table)
    W_in, b_in, W_hh, b_hh = _f(W_in), _f(b_in), _f(W_hh), _f(b_hh)
    W_s2h, b_s2h = _f(W_s2h), _f(b_s2h)
    W_act, b_act = _f(W_act), _f(b_act)
    W_push, b_push = _f(W_push), _f(b_push)
    W_out, b_out = _f(W_out), _f(b_out)
    empty_elem = _f(empty_elem)

    def kc_major(a):  # [H, X] -> [P, KC, X]
        return np.ascontiguousarray(
            a.reshape(KC, P, *a.shape[1:]).transpose(1, 0, *range(2, a.ndim + 1)))

    winT = kc_major(W_in.T)                                   # [P, KC, H]
    whhT = kc_major(W_hh.T)
    # einsum('bnd,nhd->bh'): lhsT[d, h] = W_s2h[n, h, d]
    ws2hT = np.ascontiguousarray(
        W_s2h.transpose(0, 2, 1)                              # [NS, D*E, H]
        .reshape(NS, KC, P, H).transpose(2, 0, 1, 3))         # [P, NS, KC, H]
    wactT = kc_major(W_act.transpose(2, 0, 1).reshape(H, NS * 3))
    wpushT = kc_major(W_push.transpose(2, 0, 1).reshape(H, NS * E))
    woutT = kc_major(W_out.T)                                 # [P, KC, V]
    midb = np.ascontiguousarray(
        (b_in + b_hh + b_s2h.sum(0)).reshape(KC, P).T)        # [P, KC]
    ba = np.ascontiguousarray(b_act.reshape(NS * 3, 1))
    bp = np.ascontiguousarray(b_push.reshape(NS * E, 1))
    emptyr = np.ascontiguousarray(empty_elem.reshape(1, E))
    with_bout = bool(np.any(b_out))

    shared = dict(embt=embt, winT=winT, whhT=whhT, ws2hT=ws2hT, wactT=wactT,
                  wpushT=wpushT, woutT=woutT, midb=midb, ba=ba, bp=bp,
                  emptyr=emptyr,
                  identc=np.eye(P, dtype=np.float32),
                  identstk=np.ascontiguousarray(
                      np.tile(np.eye(E, dtype=np.float32), (2, 1))))
    if with_bout:
        shared["boutd"] = np.ascontiguousarray(b_out.reshape(1, V))

    hidT_full = hidden.T                                      # [H, B] view
    in_maps = []
    for c in range(NCORES):
        b0 = c * BL
        m = dict(shared)
        m["ids"] = np.ascontiguousarray(ids[b0:b0 + BL].reshape(BL, 1))
        m["hidT"] = np.ascontiguousarray(
            hidT_full[:, b0:b0 + BL].reshape(KC, P, BL).transpose(1, 0, 2))
        m["stk"] = np.ascontiguousarray(stacks[b0:b0 + BL])
        in_maps.append(m)
    return in_maps, with_bout


def _assemble(results, ids_dtype):
    out_ls = np.concatenate([r["out_ls"] for r in results], axis=0)
    new_hidden = np.concatenate([r["out_nh"] for r in results], axis=0)
    top1 = np.concatenate([r["out_top1"] for r in results], axis=0)
    new_stacks = np.concatenate([r["out_stk"] for r in results], axis=0)
    if np.dtype(ids_dtype).itemsize == 8:
        top1 = top1.astype(np.int64)
    return out_ls, new_hidden, top1, new_stacks


def kernel(input_ids, hidden, stacks, emb_table, W_in, b_in, W_hh, b_hh,
           W_s2h, b_s2h, W_act, b_act, W_push, b_push, W_out, b_out,
           empty_elem):
    ids_dtype = np.asarray(input_ids).dtype
    in_maps, with_bout = _prep_inputs(
        input_ids, hidden, stacks, emb_table, W_in, b_in, W_hh, b_hh,
        W_s2h, b_s2h, W_act, b_act, W_push, b_push, W_out, b_out, empty_elem)
    nc = _get_program(with_bout)
    res = run_bass_kernel_spmd(nc, in_maps, list(range(NCORES)))
    return _assemble(res.results, ids_dtype)
